# revision 26
# baseline (speedup 1.0000x reference)
"""Tensor-parallel DeepSpeed-style self-attention block on 8 TRN2 NeuronCores.

v3 strategy (replicated streaming LN + XBAR z^T, transposed attention):
  - Host folds LN params into the QKV weight/bias, drops the K bias (it
    cancels in softmax), and pre-casts qkvw / attn_ow to bf16.
  - LayerNorm is recomputed on every core, streamed in 512-token
    superblocks: stats via bn_stats (DVE), z via tensor_scalar on GpSimd
    for half the tiles and Identity-activation (scale=istd, bias=-mu*istd)
    on ScalarE for the other half, then z^T is produced by the DMA XBAR
    transpose -- zero TensorE transposes, zero PSUM->SBUF copies.
  - QKV GEMM computes Q^T,K^T (transposed: [d, tok]) and V (natural).
  - Attention per (batch, head) is fully transposed:
      scores^T[k,q] = K @ Q^T, exp on ScalarE straight into p^T (causal
      diagonal via a 0/1 upper-tri multiply on DVE), rowsums via a
      ones-matmul, ctx^T[d,q] = V^T @ p^T, normalize = ScalarE copy of the
      rowsum to SBUF + GpSimd partition_broadcast + DVE reciprocal+mul.
    The kb loop is software-pipelined one iteration so the rowsum/ctx
    matmuls never wait on the exp of the chunk just produced.
  - Attention(b0) interleaves into the tail of the QKV GEMM (its ScalarE
    exp time hides under TensorE GEMM); A2A(b0) fires at the end of
    Phase A; attention(b1) + output GEMM(b0) interleave; A2A(b1) hides
    under output GEMM(b0); output GEMM is token-sharded after the A2A.
  - DMA queues: SP carries input/x/transposes/ship/out, ACT carries the
    dep-free attn_ow prefetch, collectives + broadcasts live on GpSimd,
    avoiding head-of-line blocking of compute-critical DMAs.
"""

import sys

if "/opt/trn_rl_repo" not in sys.path:
    sys.path.insert(0, "/opt/trn_rl_repo")

# --- shim antenv.axon_hooks (missing in this image) so trace=True can NTFF-profile ---
import types, ctypes, contextlib


def _make_ntff_hook(so_path="/opt/axon/libaxon_pjrt.so"):
    try:
        lib = ctypes.CDLL(so_path)
    except OSError:
        return None
    if not hasattr(lib, "axon_start_nrt_profile"):
        return None
    lib.axon_start_nrt_profile.argtypes = [ctypes.POINTER(ctypes.c_int64), ctypes.c_size_t]
    lib.axon_start_nrt_profile.restype = ctypes.c_int64
    lib.axon_stop_nrt_profile.argtypes = [ctypes.c_char_p]
    lib.axon_stop_nrt_profile.restype = ctypes.c_int64

    @contextlib.contextmanager
    def _hook(output_dir, device_ids):
        import jax

        jax.devices()
        if device_ids:
            ids = (ctypes.c_int64 * len(device_ids))(*device_ids)
            rc = lib.axon_start_nrt_profile(ids, len(device_ids))
        else:
            rc = lib.axon_start_nrt_profile(None, 0)
        if rc != 0:
            raise RuntimeError(f"axon_start_nrt_profile rc={rc}")
        try:
            yield
        finally:
            n = lib.axon_stop_nrt_profile(str(output_dir).encode())
            if n < 0:
                raise RuntimeError(f"axon_stop_nrt_profile rc={n}")

    return _hook


if "antenv.axon_hooks" not in sys.modules:
    _m = types.ModuleType("antenv.axon_hooks")
    _m.get_axon_ntff_profile_hook = lambda: _make_ntff_hook()
    sys.modules["antenv.axon_hooks"] = _m
# --- end shim ---

import numpy as np
import ml_dtypes  # noqa: F401  (bf16 numpy dtype registration)

from concourse import bacc, tile, mybir
from concourse.masks import make_upper_triangular

B, S, HID = 2, 2048, 2048
HEADS = 16
HD = 128
T = B * S
N_CORES = 8
HPC = HEADS // N_CORES  # 2 heads per core
EPS = 1e-6
SCALE = 1.0 / float(np.sqrt(HD))

F32 = mybir.dt.float32
BF16 = mybir.dt.bfloat16

SB = 512  # tokens per LN/QKV superblock
N_SB = T // SB  # 8
N_CC = HID // 128  # 16 contraction chunks
TOK_SHARD = S // N_CORES  # 256 tokens per (batch, core) after A2A
QC = 512  # attention q-chunk width


def _build(apply_mask: bool):
    nc = bacc.Bacc("TRN2", target_bir_lowering=False, debug=False, num_devices=N_CORES)

    inp = nc.dram_tensor("input", [T, HID], F32, kind="ExternalInput").ap()
    wq = nc.dram_tensor("qkvw", [HID, 3 * HPC * HD], BF16, kind="ExternalInput").ap()
    qb = nc.dram_tensor("qbias", [1, HPC * HD], BF16, kind="ExternalInput").ap()
    vb = nc.dram_tensor("vbias", [1, HPC * HD], BF16, kind="ExternalInput").ap()
    owt = nc.dram_tensor("ow", [HID, HID], BF16, kind="ExternalInput").ap()
    out = nc.dram_tensor("out", [B * TOK_SHARD, HID], F32, kind="ExternalOutput").ap()
    if apply_mask:
        imask = nc.dram_tensor("imask", [128, B * (S // 128)], F32, kind="ExternalInput").ap()

    cc_in = [nc.dram_tensor(f"cc_in{b}", [N_CORES, HPC * HD, TOK_SHARD], BF16).ap() for b in range(B)]
    cc_out = [nc.dram_tensor(f"cc_out{b}", [N_CORES, HPC * HD, TOK_SHARD], BF16).ap() for b in range(B)]

    with tile.TileContext(nc) as tc:
        with tc.tile_pool(name="persist", bufs=1) as pers:
            ones1 = pers.tile([128, 1], BF16)
            nc.gpsimd.memset(ones1[:], 1.0)
            onesr = pers.tile([1, 128], BF16)
            nc.gpsimd.memset(onesr[:], 1.0)
            ones512 = pers.tile([1, SB], BF16)
            nc.gpsimd.memset(ones512[:], 1.0)
            eps_t = pers.tile([128, 1], F32)
            nc.gpsimd.memset(eps_t[:], EPS)
            qb_sb = pers.tile([1, HPC * HD], BF16)
            nc.scalar.dma_start(out=qb_sb[:], in_=qb[:])
            vb_sb = pers.tile([1, HPC * HD], BF16)
            nc.scalar.dma_start(out=vb_sb[:], in_=vb[:])
            trif = pers.tile([128, 128], F32)
            make_upper_triangular(nc, trif[:], val=1.0, diag=True)
            tri01 = pers.tile([128, 128], BF16)
            nc.vector.tensor_copy(tri01[:], trif[:])
            if apply_mask:
                msk = pers.tile([128, B * (S // 128)], F32)
                nc.scalar.dma_start(out=msk[:], in_=imask[:])

            qT = pers.tile([128, HPC, T], BF16)  # [d, head, tok]
            kT = pers.tile([128, HPC, T], BF16)
            v_sb = pers.tile([128, T // 128, HPC * HD], BF16)  # [tok128, blk, hcol]

            # ---------- attention emitter (transposed, sw-pipelined) ----------
            def attn_qc(b, qc, ps_mm, ps_ctx, ps_rs, ppT, prb, prs, ctxT):
                nkb = 4 * qc + 4
                ctx_ps = [ps_ctx.tile([128, QC], F32, tag="ctx", name=f"ctx{h}") for h in range(HPC)]
                rs_ps = [ps_rs.tile([1, QC], F32, tag="rs", name=f"rs{h}") for h in range(HPC)]
                kbs = list(range(nkb - 1, -1, -1))
                pend = {}

                def emit_sc(kb):
                    c0 = max(0, (kb - 4 * qc) * 128)
                    w = QC - c0
                    for h in range(HPC):
                        sc = ps_mm.tile([128, QC], F32, tag="mm", name="sc")
                        nc.tensor.matmul(
                            sc[:, :w],
                            kT[:, h, b * S + kb * 128 : b * S + kb * 128 + 128],
                            qT[:, h, b * S + qc * QC + c0 : b * S + qc * QC + c0 + w],
                            start=True,
                            stop=True,
                        )
                        pt = ppT.tile([128, QC], BF16, tag="pt", name="pt")
                        bias = msk[:, b * 16 + kb : b * 16 + kb + 1] if apply_mask else 0.0
                        nc.scalar.activation(
                            pt[:, :w], sc[:, :w], mybir.ActivationFunctionType.Exp,
                            scale=SCALE, bias=bias,
                        )
                        if kb >= 4 * qc:  # causal diagonal block
                            nc.vector.tensor_mul(pt[:, 0:128], pt[:, 0:128], tri01[:])
                        pend[(h, kb)] = (pt, c0, w)

                def emit_consume(kb):
                    for h in range(HPC):
                        pt, c0, w = pend.pop((h, kb))
                        nc.tensor.matmul(
                            rs_ps[h][0:1, c0:QC], ones1[:], pt[:, :w],
                            start=(kb == kbs[0]), stop=(kb == 0),
                        )
                        nc.tensor.matmul(
                            ctx_ps[h][:, c0:QC],
                            v_sb[:, b * 16 + kb, h * HD : (h + 1) * HD],
                            pt[:, :w],
                            start=(kb == kbs[0]), stop=(kb == 0),
                        )

                for i, kb in enumerate(kbs):
                    emit_sc(kb)
                    if i > 0:
                        emit_consume(kbs[i - 1])
                emit_consume(kbs[-1])

                for h in range(HPC):
                    rsb = prs.tile([1, QC], F32, tag="rsb", name="rsb")
                    nc.scalar.copy(rsb[:], rs_ps[h][:])
                    rfa = prs.tile([1, QC], F32, tag="rfa", name="rfa")
                    nc.vector.reciprocal_approx_fast(out=rfa[:], in_=rsb[:])
                    rbc = prb.tile([128, QC], F32, tag="rbc", name="rbc")
                    nc.gpsimd.partition_broadcast(rbc[:], rfa[:])
                    nc.vector.tensor_mul(
                        ctxT[:, h, qc * QC : (qc + 1) * QC], ctx_ps[h][:], rbc[:]
                    )

            def ship_ctx(b, ctxT):
                for j in range(N_CORES):
                    nc.sync.dma_start(
                        out=cc_in[b][j].rearrange("(h d) w -> d h w", d=128),
                        in_=ctxT[:, :, j * TOK_SHARD : (j + 1) * TOK_SHARD],
                    )
                nc.gpsimd.collective_compute(
                    "AllToAll",
                    mybir.AluOpType.bypass,
                    replica_groups=[list(range(N_CORES))],
                    ins=[cc_in[b][:]],
                    outs=[cc_out[b][:]],
                )

            # ---------------- Phase A + attention(b0) interleaved ----------------
            with (
                tc.tile_pool(name="pb_pT", bufs=6) as ppT,
                tc.tile_pool(name="pb_cT", bufs=2) as pcT,
                tc.tile_pool(name="pb_rb", bufs=2) as prb,
                tc.tile_pool(name="pb_rs_sb", bufs=2) as prs,
                tc.tile_pool(name="ps_mm", bufs=4, space="PSUM") as ps_mm,
                tc.tile_pool(name="ps_ctx", bufs=2, space="PSUM") as ps_ctx,
                tc.tile_pool(name="ps_rs", bufs=2, space="PSUM") as ps_rs,
            ):
                ctxT0 = pcT.tile([128, HPC, S], BF16, tag="ctxT", name="ctxT0")
                with (
                    tc.tile_pool(name="pa_w", bufs=1) as paw,
                    tc.tile_pool(name="pa_x", bufs=5) as px,
                    tc.tile_pool(name="pa_st", bufs=6) as pst,
                    tc.tile_pool(name="pa_z", bufs=4) as pz,
                    tc.tile_pool(name="pa_zT", bufs=2) as pzT,
                ):
                    w_sb = paw.tile([128, N_CC, 3 * HPC * HD], BF16)

                    def ln_sb(sb):
                        """LN + XBAR transpose of superblock sb -> zT tile."""
                        zT = pzT.tile([128, N_CC, SB], BF16, tag="zT", name="zT")
                        xts = []
                        for tb in range(4):
                            r0 = sb * SB + tb * 128
                            x_t = px.tile([128, HID], F32, tag="x", name="x_t")
                            nc.sync.dma_start(out=x_t[:], in_=inp[r0 : r0 + 128, :])
                            xts.append(x_t)
                        zts = []
                        for tb in range(4):
                            x_t = xts[tb]
                            bn = pst.tile([128, 4, 6], F32, tag="bn", name="bn")
                            for c4 in range(4):
                                nc.vector.bn_stats(bn[:, c4, :], x_t[:, c4 * 512 : (c4 + 1) * 512])
                            mv = pst.tile([128, 2], F32, tag="mv", name="mv")
                            nc.vector.bn_aggr(mv[:], bn[:])
                            sd = pst.tile([128, 1], F32, tag="sd", name="sd")
                            nc.scalar.activation(
                                sd[:], mv[:, 1:2], mybir.ActivationFunctionType.Sqrt, bias=eps_t[:]
                            )
                            istd = pst.tile([128, 1], F32, tag="istd", name="istd")
                            nc.vector.reciprocal(istd[:], sd[:])
                            z_t = pz.tile([128, HID], BF16, tag="z", name="z_t")
                            if tb % 2 == 0:
                                nc.vector.tensor_scalar(
                                    out=z_t[:],
                                    in0=x_t[:],
                                    scalar1=mv[:, 0:1],
                                    scalar2=istd[:],
                                    op0=mybir.AluOpType.subtract,
                                    op1=mybir.AluOpType.mult,
                                )
                            else:
                                mi = pst.tile([128, 1], F32, tag="mi", name="mi")
                                nc.vector.tensor_scalar(
                                    out=mi[:],
                                    in0=mv[:, 0:1],
                                    scalar1=istd[:],
                                    scalar2=-1.0,
                                    op0=mybir.AluOpType.mult,
                                    op1=mybir.AluOpType.mult,
                                )
                                nc.scalar.activation(
                                    z_t[:], x_t[:], mybir.ActivationFunctionType.Identity,
                                    scale=istd[:], bias=mi[:],
                                )
                            zts.append(z_t)
                        for tb in range(4):
                            nc.scalar.dma_start_transpose(
                                out=zT[:, :, tb * 128 : (tb + 1) * 128], in_=zts[tb][:]
                            )
                        return zT

                    def qkv_sb(sb, zT):
                        col0 = sb * SB
                        for h in range(HPC):
                            psq = ps_mm.tile([128, SB], F32, tag="mm", name="psq")
                            for cc in range(N_CC):
                                nc.tensor.matmul(
                                    psq[:],
                                    w_sb[:, cc, h * HD : (h + 1) * HD],
                                    zT[:, cc, :],
                                    start=(cc == 0),
                                    stop=False,
                                )
                            nc.tensor.matmul(
                                psq[:], qb_sb[:, h * HD : (h + 1) * HD], ones512[:],
                                start=False, stop=True,
                            )
                            if h == 0:
                                nc.scalar.copy(qT[:, h, col0 : col0 + SB], psq[:])
                            else:
                                nc.vector.tensor_copy(qT[:, h, col0 : col0 + SB], psq[:])
                            psk = ps_mm.tile([128, SB], F32, tag="mm", name="psk")
                            for cc in range(N_CC):
                                nc.tensor.matmul(
                                    psk[:],
                                    w_sb[:, cc, HPC * HD + h * HD : HPC * HD + (h + 1) * HD],
                                    zT[:, cc, :],
                                    start=(cc == 0),
                                    stop=(cc == N_CC - 1),
                                )
                            if h == 0:
                                nc.vector.tensor_copy(kT[:, h, col0 : col0 + SB], psk[:])
                            else:
                                nc.scalar.copy(kT[:, h, col0 : col0 + SB], psk[:])
                        for tb2 in range(4):
                            psv = ps_mm.tile([128, HPC * HD], F32, tag="mm", name="psv")
                            for cc in range(N_CC):
                                nc.tensor.matmul(
                                    psv[:],
                                    zT[:, cc, tb2 * 128 : (tb2 + 1) * 128],
                                    w_sb[:, cc, 2 * HPC * HD :],
                                    start=(cc == 0),
                                    stop=False,
                                )
                            nc.tensor.matmul(
                                psv[:], onesr[:], vb_sb[:], start=False, stop=True
                            )
                            if tb2 % 2 == 0:
                                nc.scalar.copy(v_sb[:, sb * 4 + tb2, :], psv[:])
                            else:
                                nc.vector.tensor_copy(v_sb[:, sb * 4 + tb2, :], psv[:])

                    nc.scalar.dma_start(out=w_sb[:], in_=wq.rearrange("(c p) f -> p c f", p=128))
                    zT = ln_sb(0)
                    qkv_sb(0, zT)
                    for sb in range(1, 4):
                        zT = ln_sb(sb)
                        qkv_sb(sb, zT)
                    # front-load attention(b0) so A2A(b0) fires before the end of
                    # Phase A and hides under attention(b1)
                    zT = ln_sb(4)
                    qkv_sb(4, zT)
                    attn_qc(0, 0, ps_mm, ps_ctx, ps_rs, ppT, prb, prs, ctxT0)
                    zT = ln_sb(5)
                    qkv_sb(5, zT)
                    attn_qc(0, 1, ps_mm, ps_ctx, ps_rs, ppT, prb, prs, ctxT0)
                    zT7 = ln_sb(7)  # sb7 LN chain runs under sb6/qc2 compute
                    zT = ln_sb(6)
                    qkv_sb(6, zT)
                    attn_qc(0, 2, ps_mm, ps_ctx, ps_rs, ppT, prb, prs, ctxT0)
                    qkv_sb(7, zT7)
                    attn_qc(0, 3, ps_mm, ps_ctx, ps_rs, ppT, prb, prs, ctxT0)
                    ship_ctx(0, ctxT0)

                # ---------- Phase B: attention(b1) + output GEMMs ----------
                with (
                    tc.tile_pool(name="pb_ow", bufs=1) as pow_,
                    tc.tile_pool(name="pb_cf", bufs=2) as pcf,
                    tc.tile_pool(name="pb_o", bufs=2) as po,
                ):
                    ow_sb = pow_.tile([128, N_CC, HID], BF16)
                    for g in range(4):
                        nc.scalar.dma_start(
                            out=ow_sb[:, g * 4 : (g + 1) * 4, :],
                            in_=owt[g * 512 : (g + 1) * 512, :].rearrange(
                                "(c p) f -> p c f", p=128
                            ),
                        )
                    cf0 = pcf.tile([128, N_CC, TOK_SHARD], BF16, tag="cf", name="cf0")
                    nc.sync.dma_start(
                        out=cf0[:], in_=cc_out[0].rearrange("j (h d) w -> d (j h) w", d=128)
                    )

                    def outg_tb(b, cf, tb):
                        o_t = po.tile([128, HID], F32, tag="o", name="o_t")
                        for nb in range(4):
                            pso = ps_mm.tile([128, 512], F32, tag="mm", name="pso")
                            for cc in range(N_CC):
                                nc.tensor.matmul(
                                    pso[:],
                                    cf[:, cc, tb * 128 : (tb + 1) * 128],
                                    ow_sb[:, cc, nb * 512 : (nb + 1) * 512],
                                    start=(cc == 0),
                                    stop=(cc == N_CC - 1),
                                )
                            if nb % 2 == 0:
                                nc.scalar.copy(o_t[:, nb * 512 : (nb + 1) * 512], pso[:])
                            else:
                                nc.vector.tensor_copy(o_t[:, nb * 512 : (nb + 1) * 512], pso[:])
                        nc.sync.dma_start(
                            out=out[b * TOK_SHARD + tb * 128 : b * TOK_SHARD + (tb + 1) * 128, :],
                            in_=o_t[:],
                        )

                    ctxT1 = pcT.tile([128, HPC, S], BF16, tag="ctxT", name="ctxT1")
                    attn_qc(1, 0, ps_mm, ps_ctx, ps_rs, ppT, prb, prs, ctxT1)
                    attn_qc(1, 1, ps_mm, ps_ctx, ps_rs, ppT, prb, prs, ctxT1)
                    attn_qc(1, 2, ps_mm, ps_ctx, ps_rs, ppT, prb, prs, ctxT1)
                    attn_qc(1, 3, ps_mm, ps_ctx, ps_rs, ppT, prb, prs, ctxT1)
                    ship_ctx(1, ctxT1)
                    # output GEMM(b0) fills the PE while A2A(b1) is in flight
                    outg_tb(0, cf0, 0)
                    cf1 = pcf.tile([128, N_CC, TOK_SHARD], BF16, tag="cf", name="cf1")
                    nc.sync.dma_start(
                        out=cf1[:], in_=cc_out[1].rearrange("j (h d) w -> d (j h) w", d=128)
                    )
                    outg_tb(0, cf0, 1)
                    outg_tb(1, cf1, 0)
                    outg_tb(1, cf1, 1)

    nc.compile()
    return nc


_CACHE = {}


def _get_nc(apply_mask: bool):
    if apply_mask not in _CACHE:
        _CACHE[apply_mask] = _build(apply_mask)
    return _CACHE[apply_mask]


def _prep_in_maps(input, input_mask, norm_w, norm_b, attn_qkvw, attn_qkvb, attn_ow):
    bf16 = ml_dtypes.bfloat16
    x = np.ascontiguousarray(np.asarray(input, dtype=np.float32).reshape(T, HID))
    w = np.asarray(attn_qkvw, dtype=np.float32)
    nw = np.asarray(norm_w, dtype=np.float32)
    nb = np.asarray(norm_b, dtype=np.float32)
    qb_ = np.asarray(attn_qkvb, dtype=np.float32)
    ow = np.ascontiguousarray(np.asarray(attn_ow, dtype=np.float32).astype(bf16))
    mask = np.asarray(input_mask, dtype=np.float32).reshape(B, S)

    w_eff = nw[:, None] * w  # fold LN gamma into QKV weight
    b_eff = nb @ w + qb_  # fold LN beta into QKV bias

    apply_mask = bool(np.any(mask != 0.0))
    if apply_mask:
        # per-key layout: [128 partitions (k within block), B * 16 key-blocks]
        mprep = np.ascontiguousarray(
            mask.reshape(B, S // 128, 128).transpose(2, 0, 1).reshape(128, B * (S // 128))
        )
    in_maps = []
    for i in range(N_CORES):
        cols = []
        for part in range(3):  # q, k, v column shards for this core's heads
            c0 = part * HID + i * HPC * HD
            cols.append(w_eff[:, c0 : c0 + HPC * HD])
        wqkv_i = np.ascontiguousarray(np.concatenate(cols, axis=1).astype(bf16))

        qb_i = np.ascontiguousarray(
            b_eff[i * HPC * HD : (i + 1) * HPC * HD].reshape(1, HPC * HD).astype(bf16)
        )
        vb_i = np.ascontiguousarray(
            b_eff[2 * HID + i * HPC * HD : 2 * HID + (i + 1) * HPC * HD]
            .reshape(1, HPC * HD)
            .astype(bf16)
        )
        m = {
            "input": x,
            "qkvw": wqkv_i,
            "qbias": qb_i,
            "vbias": vb_i,
            "ow": ow,
        }
        if apply_mask:
            m["imask"] = mprep
        in_maps.append(m)
    return in_maps, apply_mask


def _run(inputs: dict, trace: bool = False):
    from concourse.bass_utils import run_bass_kernel_spmd

    in_maps, apply_mask = _prep_in_maps(**inputs)
    nc = _get_nc(apply_mask)
    res = run_bass_kernel_spmd(nc, in_maps, list(range(N_CORES)), trace=trace)
    out = np.empty((B, S, HID), dtype=np.float32)
    for j in range(N_CORES):
        o = res.results[j]["out"]
        for b in range(B):
            out[b, j * TOK_SHARD : (j + 1) * TOK_SHARD] = o[b * TOK_SHARD : (b + 1) * TOK_SHARD]
    return out, res


def kernel(**inputs) -> np.ndarray:
    out, _ = _run(inputs, trace=False)
    return out
